# revision 9
# baseline (speedup 1.0000x reference)
"""Multi-head self-attention with RoPE, sharded over 8 TRN2 NeuronCores.

Sharding: tensor-parallel over heads (2 heads/core) for QKV projections and
attention; an AllToAll redistributes attention outputs from head-sharded to
sequence-sharded so each core computes 1/8 of the output projection rows.

Device-side layout choices (host pre-stages everything):
- x is passed transposed (xt = x.T) so projection matmuls contract naturally.
- Wq/Wk rows are pair-permuted (evens then odds per head) so RoPE becomes
  rotate-half form; the 1/sqrt(hd) score scale is folded into Wq.
- Scores are computed transposed (S^T = K @ Q^T, keys on partitions) so the
  softmax denominator comes free from an ones-column appended to V, and P^T
  feeds the PV matmul with no on-device transpose of P.
- All matmuls run as float32r (full PE rate, ~fp22 multiply precision).

Hardcoded problem shape: B=1, S=4096, D=1024, H=16, hd=64, theta=10000.
"""

import math

import numpy as np

import concourse.bass as bass
import concourse.mybir as mybir
import concourse.tile as tile
from concourse import bacc
from concourse.bass_utils import run_bass_kernel_spmd

N_CORES = 8
D_MODEL = 1024
NUM_HEADS = 16
HEAD_DIM = 64
THETA = 10000.0
P = 128  # partitions; also = 2 heads x 64 dims per core
KD = D_MODEL // 128  # 8 contraction tiles for the projections

F32 = mybir.dt.float32
F32R = mybir.dt.float32r
EXP = mybir.ActivationFunctionType.Exp


def build(seq: int):
    """Build the SPMD Bass program for sequence length `seq`."""
    CH = min(512, seq)          # free-dim chunk for matmuls / PSUM banks
    NCH = seq // CH             # number of seq chunks
    KB = seq // 128             # key blocks
    KBC = CH // 128             # key blocks per chunk (4 at CH=512)
    SW = seq // N_CORES         # per-core output seq shard
    SPC = CH // SW if CH >= SW else 1  # a2a shards per chunk

    nc = bacc.Bacc("TRN2", num_devices=N_CORES)

    xt = nc.dram_tensor("xt", [D_MODEL, seq], F32R, kind="ExternalInput")
    wq = nc.dram_tensor("wq", [P, D_MODEL], F32R, kind="ExternalInput")
    wk = nc.dram_tensor("wk", [P, D_MODEL], F32R, kind="ExternalInput")
    wv = nc.dram_tensor("wv", [P, D_MODEL], F32R, kind="ExternalInput")
    wo = nc.dram_tensor("wo", [P, KD * D_MODEL], F32R, kind="ExternalInput")
    ctab = nc.dram_tensor("ctab", [P, seq], F32, kind="ExternalInput")
    stab = nc.dram_tensor("stab", [P, seq], F32, kind="ExternalInput")
    trimask = nc.dram_tensor("trimask", [P, 128], F32, kind="ExternalInput")
    trimask2 = nc.dram_tensor("trimask2", [P, 256], F32, kind="ExternalInput")
    ident = nc.dram_tensor("ident", [P, 128], F32, kind="ExternalInput")
    onesd = nc.dram_tensor("ones", [P, KB], F32, kind="ExternalInput")
    out_d = nc.dram_tensor("out", [D_MODEL, SW], F32, kind="ExternalOutput")

    with tile.TileContext(nc) as tc:
        with (
            tc.tile_pool(name="const", bufs=1) as cpool,
            tc.tile_pool(name="mats", bufs=1) as mpool,
            tc.tile_pool(name="xt", bufs=2) as xpool,
            tc.tile_pool(name="sc", bufs=2) as spool,
            tc.tile_pool(name="pt", bufs=4) as ptpool,
            tc.tile_pool(name="wo", bufs=3) as wopool,
            tc.tile_pool(name="ps", bufs=2, space="PSUM") as pspool,
            tc.tile_pool(name="pss", bufs=3, space="PSUM") as psspool,
            tc.tile_pool(name="dram", bufs=1, space="DRAM") as dpool,
        ):
            # ---- constants ----
            w_sb = {}
            for name, src in (("q", wq), ("k", wk), ("v", wv)):
                t = cpool.tile([P, D_MODEL], F32R, tag=f"w{name}")
                nc.sync.dma_start(out=t[:], in_=src[:])
                w_sb[name] = t
            ct = cpool.tile([P, seq], F32, tag="ct")
            nc.sync.dma_start(out=ct[:], in_=ctab[:])
            st = cpool.tile([P, seq], F32, tag="st")
            nc.sync.dma_start(out=st[:], in_=stab[:])
            tri = cpool.tile([P, 128], F32, tag="tri")
            nc.sync.dma_start(out=tri[:], in_=trimask[:])
            tri2 = cpool.tile([P, 256], F32, tag="tri2")
            nc.sync.dma_start(out=tri2[:], in_=trimask2[:])
            idn = cpool.tile([P, 128], F32, tag="idn")
            nc.sync.dma_start(out=idn[:], in_=ident[:])
            ones = cpool.tile([P, KB], F32, tag="ones")
            nc.sync.dma_start(out=ones[:], in_=onesd[:])

            # ---- persistent matrices ----
            qT = mpool.tile([P, seq], F32R, tag="qT")   # rows: 2 heads x 64 dims
            kT = mpool.tile([P, seq], F32R, tag="kT")
            vnat = mpool.tile([P, KB * 130], F32R, tag="vnat")  # [Vh0|1|Vh1|1]/kb

            # fill the per-key-block ones columns (cols 64 and 129 of each
            # 130-wide block) via strided DVE copies so the output is f32r
            vv = vnat[:].rearrange("p (k c) -> p k c", c=130)
            oo = ones[:].rearrange("p (k c) -> p k c", c=1)
            nc.vector.tensor_copy(vv[:, :, 64:65], oo)
            nc.vector.tensor_copy(vv[:, :, 129:130], oo)

            # ---- phase 1: projections (+ rope + V-transpose interleaved) ----
            for sc in range(NCH):
                sl = bass.ts(sc, CH)
                xts = []
                for k in range(KD):
                    t = xpool.tile([P, CH], F32R, tag=f"xt{k}")
                    nc.sync.dma_start(out=t[:], in_=xt[128 * k:128 * (k + 1), sl])
                    xts.append(t)
                vt_c = spool.tile([P, CH], F32, tag="vt")
                for name, dst in (("q", qT[:, sl]), ("k", kT[:, sl]),
                                  ("v", vt_c[:])):
                    ps = pspool.tile([P, CH], F32, tag="mm")
                    for k in range(KD):
                        nc.tensor.matmul(
                            ps[:],
                            w_sb[name][:, bass.ts(k, 128)],
                            xts[k][:],
                            start=(k == 0),
                            stop=(k == KD - 1),
                        )
                    nc.vector.tensor_copy(dst, ps[:])

                # rope on this chunk of qT and kT (in place)
                for mat in (qT, kT):
                    sw = spool.tile([P, CH], F32, tag="swap")
                    for h in (0, 1):
                        for half in (0, 1):
                            d0 = 64 * h + 32 * half
                            s0 = 64 * h + 32 * (1 - half)
                            nc.vector.tensor_copy(
                                sw[d0:d0 + 32, :], mat[s0:s0 + 32, sl]
                            )
                    tm = spool.tile([P, CH], F32, tag="tmp")
                    nc.vector.tensor_mul(tm[:], sw[:], st[:, sl])
                    nc.vector.tensor_mul(mat[:, sl], mat[:, sl], ct[:, sl])
                    nc.vector.tensor_add(mat[:, sl], mat[:, sl], tm[:])

                # V transpose for this chunk's key blocks
                for j in range(KBC):
                    kb = sc * KBC + j
                    pst = psspool.tile([P, CH], F32, tag="s")
                    nc.tensor.transpose(
                        pst[:, 0:128], vt_c[:, bass.ts(j, 128)], idn[:]
                    )
                    nc.vector.tensor_copy(
                        vnat[:, 130 * kb:130 * kb + 64], pst[:, 0:64]
                    )
                    nc.vector.tensor_copy(
                        vnat[:, 130 * kb + 65:130 * kb + 129], pst[:, 64:128]
                    )

            # ---- phase 2: attention (S^T -> exp -> PV with ones column) ----
            a2a_in = dpool.tile([N_CORES, P, SW], F32R, tag="a2a_in")
            a2a_out = dpool.tile([N_CORES, P, SW], F32R, tag="a2a_out")

            for qc in range(NCH):
                kbmax = (qc + 1) * KBC
                psu = {}
                for h in (0, 1):
                    psu[h] = pspool.tile([65, CH], F32, tag="u", name=f"psu{h}")
                for kb in range(kbmax):
                    j = kb - (kbmax - KBC)  # diag index if >= 0
                    if j < 0:
                        col_off = 0
                    else:
                        col_off = min(128 * j, CH - 256) if CH >= 256 else 0
                    for h in (0, 1):
                        pss = psspool.tile([P, CH], F32, tag="s")
                        nc.tensor.matmul(
                            pss[:, col_off:CH],
                            kT[64 * h:64 * (h + 1), bass.ts(kb, 128)]
                            ,
                            qT[64 * h:64 * (h + 1),
                               CH * qc + col_off:CH * (qc + 1)],
                            start=True,
                            stop=True,
                        )
                        pt = ptpool.tile([P, CH], F32R, tag="pt")
                        nc.scalar.activation(
                            pt[:, col_off:CH], pss[:, col_off:CH], EXP
                        )
                        if j >= 0:
                            tw0 = 128 * j  # triangle window start
                            if tw0 > col_off:
                                nc.vector.tensor_mul(
                                    pt[:, col_off:tw0 + 128],
                                    pt[:, col_off:tw0 + 128],
                                    tri2[:],
                                )
                            else:
                                nc.vector.tensor_mul(
                                    pt[:, tw0:tw0 + 128],
                                    pt[:, tw0:tw0 + 128],
                                    tri[:],
                                )
                        nc.tensor.matmul(
                            psu[h][:, col_off:CH],
                            vnat[:, 130 * kb + 65 * h:130 * kb + 65 * (h + 1)]
                            ,
                            pt[:, col_off:CH],
                            start=(kb == 0),
                            stop=(kb == kbmax - 1),
                        )
                # epilogue: extract U and sums, normalize, stage a2a input
                for h in (0, 1):
                    ut = spool.tile([64, CH], F32R, tag=f"ut{h}")
                    nc.vector.tensor_copy(ut[:], psu[h][0:64, :])
                    ss = spool.tile([1, CH], F32, tag=f"ss{h}")
                    nc.vector.tensor_copy(ss[:], psu[h][64:65, :])
                    rs = spool.tile([1, CH], F32, tag=f"rs{h}")
                    nc.vector.reciprocal(rs[:], ss[:])
                    rb = spool.tile([64, CH], F32, tag=f"rb{h}")
                    nc.gpsimd.partition_broadcast(rb[:], rs[:])
                    nc.vector.tensor_mul(ut[:], ut[:], rb[:])
                    for jj in range(SPC):
                        shard = qc * SPC + jj
                        nc.sync.dma_start(
                            out=a2a_in[shard, 64 * h:64 * (h + 1), :],
                            in_=ut[:, SW * jj:SW * (jj + 1)],
                        )

            # ---- phase 3: A2A + output projection ----
            nc.gpsimd.collective_compute(
                "AllToAll",
                mybir.AluOpType.bypass,
                replica_groups=[list(range(N_CORES))],
                ins=[a2a_in.opt()],
                outs=[a2a_out.opt()],
            )
            ats = []
            for i in range(N_CORES):
                at = xpool.tile([P, SW], F32R, tag=f"xt{i}")
                nc.sync.dma_start(out=at[:], in_=a2a_out[i])
                ats.append(at)
            for e in range(KD):
                wot = wopool.tile([P, D_MODEL], F32R, tag="wo")
                nc.sync.dma_start(out=wot[:], in_=wo[:, bass.ts(e, D_MODEL)])
                pso = pspool.tile([P, SW], F32, tag="mm")
                for i in range(N_CORES):
                    nc.tensor.matmul(
                        pso[:],
                        wot[:, bass.ts(i, 128)],
                        ats[i][:],
                        start=(i == 0),
                        stop=(i == N_CORES - 1),
                    )
                ot = ptpool.tile([P, SW], F32, tag="pt")
                nc.vector.tensor_copy(ot[:], pso[:])
                nc.sync.dma_start(out=out_d[bass.ts(e, 128)], in_=ot[:])

    nc.finalize()
    return nc


def prepare_in_maps(in_features, token_positions, Wq, Wk, Wv, Wo, seq):
    """Host-side staging: shard/transform full inputs into per-core maps."""
    x = np.ascontiguousarray(np.asarray(in_features, dtype=np.float32)[0])
    pos = np.asarray(token_positions).reshape(-1)[:seq].astype(np.float64)

    xt = np.ascontiguousarray(x.T)  # [D, S]

    # RoPE tables in rotate-half form after pair permutation.
    inv_freq = THETA ** (-np.arange(0, HEAD_DIM, 2, dtype=np.float64) / HEAD_DIM)
    ang = pos[:, None] * inv_freq[None, :]  # [S, 32]
    cos = np.cos(ang).T.astype(np.float32)  # [32, S]
    sin = np.sin(ang).T.astype(np.float32)
    ctab = np.ascontiguousarray(np.tile(cos, (4, 1)))  # [128, S]
    stab = np.ascontiguousarray(
        np.concatenate([-sin, sin, -sin, sin], axis=0)
    ).astype(np.float32)

    perm = np.concatenate(
        [np.arange(0, HEAD_DIM, 2), np.arange(1, HEAD_DIM, 2)]
    )  # within-head: evens then odds

    tri = np.ascontiguousarray(np.triu(np.ones((128, 128), dtype=np.float32)))
    tri2 = np.ascontiguousarray(
        np.concatenate([np.zeros((128, 128), dtype=np.float32), tri], axis=1)
    )
    ident = np.eye(128, dtype=np.float32)
    ones = np.ones((128, seq // 128), dtype=np.float32)

    WoT = np.ascontiguousarray(np.asarray(Wo, dtype=np.float32).T)  # [d, e]
    wo_packed = np.empty((128, KD * D_MODEL), dtype=np.float32)
    for e in range(KD):
        for i in range(KD):
            wo_packed[:, D_MODEL * e + 128 * i: D_MODEL * e + 128 * (i + 1)] = \
                WoT[128 * i:128 * (i + 1), 128 * e:128 * (e + 1)]

    def pack_w(Wc):
        # Wc: [128 out, 1024 in] -> WT [1024, 128] -> [128, 8*128] k-tiled
        WT = np.ascontiguousarray(Wc.T)
        return np.ascontiguousarray(
            WT.reshape(KD, 128, 128).transpose(1, 0, 2).reshape(128, KD * 128)
        ).astype(np.float32)

    in_maps = []
    for c in range(N_CORES):
        rows = slice(128 * c, 128 * (c + 1))
        Wq_c = np.asarray(Wq, dtype=np.float32)[rows].reshape(2, 64, D_MODEL)
        Wq_c = (Wq_c[:, perm, :] / math.sqrt(HEAD_DIM)).reshape(128, D_MODEL)
        Wk_c = np.asarray(Wk, dtype=np.float32)[rows].reshape(2, 64, D_MODEL)
        Wk_c = Wk_c[:, perm, :].reshape(128, D_MODEL)
        Wv_c = np.asarray(Wv, dtype=np.float32)[rows]
        in_maps.append({
            "xt": xt,
            "wq": pack_w(Wq_c),
            "wk": pack_w(Wk_c),
            "wv": pack_w(Wv_c),
            "wo": wo_packed,
            "ctab": ctab,
            "stab": stab,
            "trimask": tri,
            "trimask2": tri2,
            "ident": ident,
            "ones": ones,
        })
    return in_maps


_BUILD_CACHE = {}


def _get_nc(seq):
    if seq not in _BUILD_CACHE:
        _BUILD_CACHE[seq] = build(seq)
    return _BUILD_CACHE[seq]


def postprocess(results, seq, in_dtype):
    SW = seq // N_CORES
    out = np.empty((seq, D_MODEL), dtype=np.float32)
    for c in range(N_CORES):
        out[SW * c:SW * (c + 1), :] = results[c]["out"].T
    return out.reshape(1, seq, D_MODEL).astype(in_dtype)


def kernel(in_features, token_positions, Wq, Wk, Wv, Wo):
    in_dtype = np.asarray(in_features).dtype
    B, S, D = np.asarray(in_features).shape
    assert B == 1 and D == D_MODEL

    nc = _get_nc(S)
    in_maps = prepare_in_maps(in_features, token_positions, Wq, Wk, Wv, Wo, S)
    res = run_bass_kernel_spmd(nc, in_maps, list(range(N_CORES)), trace=False)
    return postprocess(res.results, S, in_dtype)


# revision 13
# speedup vs baseline: 44.2079x; 44.2079x over previous
"""Multi-head self-attention with RoPE, sharded over 8 TRN2 NeuronCores.

Sharding: tensor-parallel over heads (2 heads/core) for QKV projections and
attention; an AllToAll redistributes attention outputs from head-sharded to
sequence-sharded so each core computes 1/8 of the output projection rows.

Device-side layout choices (host pre-stages everything):
- x is passed transposed (xt = x.T) so projection matmuls contract naturally.
- Wq/Wk rows are pair-permuted (evens then odds per head) so RoPE becomes
  rotate-half form; the 1/sqrt(hd) score scale is folded into Wq.
- Scores are computed transposed (S^T = K @ Q^T, keys on partitions) so the
  softmax denominator comes free from an ones-column appended to V, and P^T
  feeds the PV matmul with no on-device transpose of P.
- All matmuls run as float32r (full PE rate, ~fp22 multiply precision).

Hardcoded problem shape: B=1, S=4096, D=1024, H=16, hd=64, theta=10000.
"""

import math

import numpy as np

import concourse.bass as bass
import concourse.mybir as mybir
import concourse.tile as tile
from concourse import bacc
from concourse.bass_utils import run_bass_kernel_spmd

N_CORES = 8
D_MODEL = 1024
NUM_HEADS = 16
HEAD_DIM = 64
THETA = 10000.0
P = 128  # partitions; also = 2 heads x 64 dims per core
KD = D_MODEL // 128  # 8 contraction tiles for the projections

F32 = mybir.dt.float32
F32R = mybir.dt.float32r
EXP = mybir.ActivationFunctionType.Exp


def build(seq: int, p12_reps: int = 1, p3_reps: int = 1):
    """Build the SPMD Bass program for sequence length `seq`.

    p12_reps > 1 wraps phases 1+2 (projections + attention) in an on-device
    For_i loop; p3_reps > 1 unrolls phase 3 (A2A + out-proj) — both exist
    for wall-clock timing above the axon dispatch floor. Defaults give the
    normal single-shot kernel.
    """
    CH = min(512, seq)          # free-dim chunk for matmuls / PSUM banks
    NCH = seq // CH             # number of seq chunks
    KB = seq // 128             # key blocks
    KBC = CH // 128             # key blocks per chunk (4 at CH=512)
    SW = seq // N_CORES         # per-core output seq shard
    SPC = CH // SW if CH >= SW else 1  # a2a shards per chunk

    nc = bacc.Bacc("TRN2", num_devices=N_CORES)

    xt = nc.dram_tensor("xt", [D_MODEL, seq], F32R, kind="ExternalInput")
    wq = nc.dram_tensor("wq", [P, D_MODEL], F32R, kind="ExternalInput")
    wk = nc.dram_tensor("wk", [P, D_MODEL], F32R, kind="ExternalInput")
    wv = nc.dram_tensor("wv", [P, D_MODEL], F32R, kind="ExternalInput")
    wo = nc.dram_tensor("wo", [P, KD * D_MODEL], F32R, kind="ExternalInput")
    ctab = nc.dram_tensor("ctab", [P, seq], F32, kind="ExternalInput")
    stab = nc.dram_tensor("stab", [P, seq], F32, kind="ExternalInput")
    trimask = nc.dram_tensor("trimask", [P, 128], F32, kind="ExternalInput")
    trimask2 = nc.dram_tensor("trimask2", [P, 256], F32, kind="ExternalInput")
    ident = nc.dram_tensor("ident", [P, 128], F32, kind="ExternalInput")
    onesd = nc.dram_tensor("ones", [P, KB], F32, kind="ExternalInput")
    out_d = nc.dram_tensor("out", [D_MODEL, SW], F32, kind="ExternalOutput")

    with tile.TileContext(nc) as tc:
        with (
            tc.tile_pool(name="const", bufs=1) as cpool,
            tc.tile_pool(name="mats", bufs=1) as mpool,
            tc.tile_pool(name="xt", bufs=2) as xpool,
            tc.tile_pool(name="sc", bufs=2) as spool,
            tc.tile_pool(name="pt", bufs=4) as ptpool,
            tc.tile_pool(name="wo", bufs=3) as wopool,
            tc.tile_pool(name="ps", bufs=2, space="PSUM") as pspool,
            tc.tile_pool(name="pss", bufs=3, space="PSUM") as psspool,
            tc.tile_pool(name="dram", bufs=1, space="DRAM") as dpool,
        ):
            # ---- constants ----
            w_sb = {}
            for name, src in (("q", wq), ("k", wk), ("v", wv)):
                t = cpool.tile([P, D_MODEL], F32R, tag=f"w{name}")
                nc.sync.dma_start(out=t[:], in_=src[:])
                w_sb[name] = t
            ct = cpool.tile([P, seq], F32, tag="ct")
            nc.sync.dma_start(out=ct[:], in_=ctab[:])
            st = cpool.tile([P, seq], F32, tag="st")
            nc.sync.dma_start(out=st[:], in_=stab[:])
            tri = cpool.tile([P, 128], F32, tag="tri")
            nc.sync.dma_start(out=tri[:], in_=trimask[:])
            tri2 = cpool.tile([P, 256], F32, tag="tri2")
            nc.sync.dma_start(out=tri2[:], in_=trimask2[:])
            idn = cpool.tile([P, 128], F32, tag="idn")
            nc.sync.dma_start(out=idn[:], in_=ident[:])
            ones = cpool.tile([P, KB], F32, tag="ones")
            nc.sync.dma_start(out=ones[:], in_=onesd[:])

            # ---- persistent matrices ----
            qT = mpool.tile([P, seq], F32R, tag="qT")  # rows: 2 heads x 64
            kT = mpool.tile([P, seq], F32R, tag="kT")
            vnat = mpool.tile([P, KB * 130], F32R, tag="vnat")

            a2a_in = dpool.tile([N_CORES, P, SW], F32R, tag="a2a_in")
            a2a_out = dpool.tile([N_CORES, P, SW], F32R, tag="a2a_out")

            def emit_p12():
                # ones columns (cols 64 and 129 of each 130-wide block) via
                # strided DVE copies so the producer output dtype is f32r
                vv = vnat[:].rearrange("p (k c) -> p k c", c=130)
                oo = ones[:].rearrange("p (k c) -> p k c", c=1)
                nc.vector.tensor_copy(vv[:, :, 64:65], oo)
                nc.vector.tensor_copy(vv[:, :, 129:130], oo)

                # phase 1: projections + rope + V-transpose, per seq chunk
                for sc in range(NCH):
                    sl = bass.ts(sc, CH)
                    xts = []
                    for k in range(KD):
                        t = xpool.tile([P, CH], F32R, tag=f"xt{k}",
                                       name=f"xt_{sc}_{k}")
                        nc.sync.dma_start(
                            out=t[:], in_=xt[128 * k:128 * (k + 1), sl]
                        )
                        xts.append(t)
                    vt_c = spool.tile([P, CH], F32, tag="vt")
                    for name, dst in (("q", qT[:, sl]), ("k", kT[:, sl]),
                                      ("v", vt_c[:])):
                        ps = pspool.tile([P, CH], F32, tag="mm",
                                         name=f"proj_{sc}_{name}")
                        for k in range(KD):
                            nc.tensor.matmul(
                                ps[:],
                                w_sb[name][:, bass.ts(k, 128)],
                                xts[k][:],
                                start=(k == 0),
                                stop=(k == KD - 1),
                            )
                        nc.vector.tensor_copy(dst, ps[:])

                    # rope on this chunk of qT and kT (in place)
                    for mi, mat in ((0, qT), (1, kT)):
                        sw = spool.tile([P, CH], F32, tag="swap",
                                        name=f"swap_{sc}_{mi}")
                        for h in (0, 1):
                            for half in (0, 1):
                                d0 = 64 * h + 32 * half
                                s0 = 64 * h + 32 * (1 - half)
                                nc.vector.tensor_copy(
                                    sw[d0:d0 + 32, :], mat[s0:s0 + 32, sl]
                                )
                        tm = spool.tile([P, CH], F32, tag="tmp",
                                        name=f"tmp_{sc}_{mi}")
                        nc.vector.tensor_mul(tm[:], sw[:], st[:, sl])
                        nc.vector.tensor_mul(mat[:, sl], mat[:, sl],
                                             ct[:, sl])
                        nc.vector.tensor_add(mat[:, sl], mat[:, sl], tm[:])

                    # V transpose for this chunk's key blocks
                    for j in range(KBC):
                        kb = sc * KBC + j
                        pst = psspool.tile([P, CH], F32, tag="s",
                                           name=f"vtr_{kb}")
                        nc.tensor.transpose(
                            pst[:, 0:128], vt_c[:, bass.ts(j, 128)], idn[:]
                        )
                        nc.vector.tensor_copy(
                            vnat[:, 130 * kb:130 * kb + 64], pst[:, 0:64]
                        )
                        nc.vector.tensor_copy(
                            vnat[:, 130 * kb + 65:130 * kb + 129],
                            pst[:, 64:128]
                        )

                # phase 2: attention
                for qc in range(NCH):
                    kbmax = (qc + 1) * KBC
                    psu = {}
                    for h in (0, 1):
                        psu[h] = pspool.tile([65, CH], F32, tag="u",
                                             name=f"psu_{qc}_{h}")
                    for kb in range(kbmax):
                        j = kb - (kbmax - KBC)  # diag index if >= 0
                        if j < 0:
                            col_off = 0
                        else:
                            col_off = (min(128 * j, CH - 256)
                                       if CH >= 256 else 0)
                        for h in (0, 1):
                            pss = psspool.tile([P, CH], F32, tag="s",
                                               name=f"sc_{qc}_{kb}_{h}")
                            nc.tensor.matmul(
                                pss[:, col_off:CH],
                                kT[64 * h:64 * (h + 1), bass.ts(kb, 128)],
                                qT[64 * h:64 * (h + 1),
                                   CH * qc + col_off:CH * (qc + 1)],
                                start=True,
                                stop=True,
                            )
                            pt = ptpool.tile([P, CH], F32R, tag="pt",
                                             name=f"pt_{qc}_{kb}_{h}")
                            nc.scalar.activation(
                                pt[:, col_off:CH], pss[:, col_off:CH], EXP
                            )
                            if j >= 0:
                                tw0 = 128 * j  # triangle window start
                                if tw0 > col_off:
                                    nc.vector.tensor_mul(
                                        pt[:, col_off:tw0 + 128],
                                        pt[:, col_off:tw0 + 128],
                                        tri2[:],
                                    )
                                else:
                                    nc.vector.tensor_mul(
                                        pt[:, tw0:tw0 + 128],
                                        pt[:, tw0:tw0 + 128],
                                        tri[:],
                                    )
                            nc.tensor.matmul(
                                psu[h][:, col_off:CH],
                                vnat[:, 130 * kb + 65 * h:
                                     130 * kb + 65 * (h + 1)],
                                pt[:, col_off:CH],
                                start=(kb == 0),
                                stop=(kb == kbmax - 1),
                            )
                    # epilogue: extract U and sums, normalize, stage a2a in
                    for h in (0, 1):
                        ut = spool.tile([64, CH], F32R, tag=f"ut{h}",
                                        name=f"ut_{qc}_{h}")
                        nc.vector.tensor_copy(ut[:], psu[h][0:64, :])
                        ss = spool.tile([1, CH], F32, tag=f"ss{h}",
                                        name=f"ss_{qc}_{h}")
                        nc.vector.tensor_copy(ss[:], psu[h][64:65, :])
                        rs = spool.tile([1, CH], F32, tag=f"rs{h}",
                                        name=f"rs_{qc}_{h}")
                        nc.vector.reciprocal(rs[:], ss[:])
                        rb = spool.tile([64, CH], F32, tag=f"rb{h}",
                                        name=f"rb_{qc}_{h}")
                        nc.gpsimd.partition_broadcast(rb[:], rs[:])
                        nc.vector.tensor_mul(ut[:], ut[:], rb[:])
                        for jj in range(SPC):
                            shard = qc * SPC + jj
                            nc.sync.dma_start(
                                out=a2a_in[shard, 64 * h:64 * (h + 1), :],
                                in_=ut[:, SW * jj:SW * (jj + 1)],
                            )

            def emit_p3():
                nc.gpsimd.collective_compute(
                    "AllToAll",
                    mybir.AluOpType.bypass,
                    replica_groups=[list(range(N_CORES))],
                    ins=[a2a_in.opt()],
                    outs=[a2a_out.opt()],
                )
                ats = []
                for i in range(N_CORES):
                    at = xpool.tile([P, SW], F32R, tag=f"xt{i}",
                                    name=f"at_{i}")
                    nc.sync.dma_start(out=at[:], in_=a2a_out[i])
                    ats.append(at)
                for e in range(KD):
                    wot = wopool.tile([P, D_MODEL], F32R, tag="wo",
                                      name=f"wot_{e}")
                    nc.sync.dma_start(
                        out=wot[:], in_=wo[:, bass.ts(e, D_MODEL)]
                    )
                    pso = pspool.tile([P, SW], F32, tag="mm",
                                      name=f"pso_{e}")
                    for i in range(N_CORES):
                        nc.tensor.matmul(
                            pso[:],
                            wot[:, bass.ts(i, 128)],
                            ats[i][:],
                            start=(i == 0),
                            stop=(i == N_CORES - 1),
                        )
                    ot = ptpool.tile([P, SW], F32, tag="pt",
                                     name=f"ot_{e}")
                    nc.vector.tensor_copy(ot[:], pso[:])
                    nc.sync.dma_start(out=out_d[bass.ts(e, 128)], in_=ot[:])

            if p12_reps == 1:
                emit_p12()
            else:
                with tc.For_i(0, p12_reps, 1):
                    emit_p12()
            for _ in range(p3_reps):
                emit_p3()

    nc.finalize()
    return nc


def prepare_in_maps(in_features, token_positions, Wq, Wk, Wv, Wo, seq):
    """Host-side staging: shard/transform full inputs into per-core maps."""
    x = np.ascontiguousarray(np.asarray(in_features, dtype=np.float32)[0])
    pos = np.asarray(token_positions).reshape(-1)[:seq].astype(np.float64)

    xt = np.ascontiguousarray(x.T)  # [D, S]

    # RoPE tables in rotate-half form after pair permutation.
    inv_freq = THETA ** (-np.arange(0, HEAD_DIM, 2, dtype=np.float64)
                         / HEAD_DIM)
    ang = pos[:, None] * inv_freq[None, :]  # [S, 32]
    cos = np.cos(ang).T.astype(np.float32)  # [32, S]
    sin = np.sin(ang).T.astype(np.float32)
    ctab = np.ascontiguousarray(np.tile(cos, (4, 1)))  # [128, S]
    stab = np.ascontiguousarray(
        np.concatenate([-sin, sin, -sin, sin], axis=0)
    ).astype(np.float32)

    perm = np.concatenate(
        [np.arange(0, HEAD_DIM, 2), np.arange(1, HEAD_DIM, 2)]
    )  # within-head: evens then odds

    tri = np.ascontiguousarray(np.triu(np.ones((128, 128), dtype=np.float32)))
    tri2 = np.ascontiguousarray(
        np.concatenate([np.zeros((128, 128), dtype=np.float32), tri], axis=1)
    )
    ident = np.eye(128, dtype=np.float32)
    ones = np.ones((128, seq // 128), dtype=np.float32)

    WoT = np.ascontiguousarray(np.asarray(Wo, dtype=np.float32).T)  # [d, e]
    wo_packed = np.empty((128, KD * D_MODEL), dtype=np.float32)
    for e in range(KD):
        for i in range(KD):
            wo_packed[:, D_MODEL * e + 128 * i: D_MODEL * e + 128 * (i + 1)] \
                = WoT[128 * i:128 * (i + 1), 128 * e:128 * (e + 1)]

    def pack_w(Wc):
        # Wc: [128 out, 1024 in] -> WT [1024, 128] -> [128, 8*128] k-tiled
        WT = np.ascontiguousarray(Wc.T)
        return np.ascontiguousarray(
            WT.reshape(KD, 128, 128).transpose(1, 0, 2).reshape(128, KD * 128)
        ).astype(np.float32)

    in_maps = []
    for c in range(N_CORES):
        rows = slice(128 * c, 128 * (c + 1))
        Wq_c = np.asarray(Wq, dtype=np.float32)[rows].reshape(2, 64, D_MODEL)
        Wq_c = (Wq_c[:, perm, :] / math.sqrt(HEAD_DIM)).reshape(128, D_MODEL)
        Wk_c = np.asarray(Wk, dtype=np.float32)[rows].reshape(2, 64, D_MODEL)
        Wk_c = Wk_c[:, perm, :].reshape(128, D_MODEL)
        Wv_c = np.asarray(Wv, dtype=np.float32)[rows]
        in_maps.append({
            "xt": xt,
            "wq": pack_w(Wq_c),
            "wk": pack_w(Wk_c),
            "wv": pack_w(Wv_c),
            "wo": wo_packed,
            "ctab": ctab,
            "stab": stab,
            "trimask": tri,
            "trimask2": tri2,
            "ident": ident,
            "ones": ones,
        })
    return in_maps


_BUILD_CACHE = {}


def _get_nc(seq, p12_reps=1, p3_reps=1):
    key = (seq, p12_reps, p3_reps)
    if key not in _BUILD_CACHE:
        _BUILD_CACHE[key] = build(seq, p12_reps, p3_reps)
    return _BUILD_CACHE[key]


def postprocess(results, seq, in_dtype):
    SW = seq // N_CORES
    out = np.empty((seq, D_MODEL), dtype=np.float32)
    for c in range(N_CORES):
        out[SW * c:SW * (c + 1), :] = results[c]["out"].T
    return out.reshape(1, seq, D_MODEL).astype(in_dtype)


def kernel(in_features, token_positions, Wq, Wk, Wv, Wo):
    in_dtype = np.asarray(in_features).dtype
    B, S, D = np.asarray(in_features).shape
    assert B == 1 and D == D_MODEL

    nc = _get_nc(S)
    in_maps = prepare_in_maps(in_features, token_positions, Wq, Wk, Wv, Wo, S)
    res = run_bass_kernel_spmd(nc, in_maps, list(range(N_CORES)), trace=False)
    return postprocess(res.results, S, in_dtype)


# revision 15
# speedup vs baseline: 45.0651x; 1.0194x over previous
"""Multi-head self-attention with RoPE, sharded over 8 TRN2 NeuronCores.

Sharding: tensor-parallel over heads (2 heads/core) for QKV projections and
attention; an AllToAll redistributes attention outputs from head-sharded to
sequence-sharded so each core computes 1/8 of the output projection rows.

Device-side layout choices (host pre-stages everything):
- x is passed transposed (xt = x.T) so projection matmuls contract naturally.
- Wq/Wk rows are pair-permuted (evens then odds per head) so RoPE becomes
  rotate-half form; the 1/sqrt(hd) score scale is folded into Wq.
- Scores are computed transposed (S^T = K @ Q^T, keys on partitions) so the
  softmax denominator comes free from an ones-column appended to V, and P^T
  feeds the PV matmul with no on-device transpose of P.
- All matmuls run as float32r (full PE rate, ~fp22 multiply precision).

Hardcoded problem shape: B=1, S=4096, D=1024, H=16, hd=64, theta=10000.
"""

import math

import numpy as np

import concourse.bass as bass
import concourse.mybir as mybir
import concourse.tile as tile
from concourse import bacc
from concourse.bass_utils import run_bass_kernel_spmd

N_CORES = 8
D_MODEL = 1024
NUM_HEADS = 16
HEAD_DIM = 64
THETA = 10000.0
P = 128  # partitions; also = 2 heads x 64 dims per core
KD = D_MODEL // 128  # 8 contraction tiles for the projections

F32 = mybir.dt.float32
F32R = mybir.dt.float32r
EXP = mybir.ActivationFunctionType.Exp


def build(seq: int, p12_reps: int = 1, p3_reps: int = 1):
    """Build the SPMD Bass program for sequence length `seq`.

    p12_reps > 1 wraps phases 1+2 (projections + attention) in an on-device
    For_i loop; p3_reps > 1 unrolls phase 3 (A2A + out-proj) — both exist
    for wall-clock timing above the axon dispatch floor. Defaults give the
    normal single-shot kernel.
    """
    CH = min(512, seq)          # free-dim chunk for matmuls / PSUM banks
    NCH = seq // CH             # number of seq chunks
    KB = seq // 128             # key blocks
    KBC = CH // 128             # key blocks per chunk (4 at CH=512)
    SW = seq // N_CORES         # per-core output seq shard
    SPC = CH // SW if CH >= SW else 1  # a2a shards per chunk

    nc = bacc.Bacc("TRN2", num_devices=N_CORES)

    xt = nc.dram_tensor("xt", [D_MODEL, seq], F32R, kind="ExternalInput")
    wq = nc.dram_tensor("wq", [P, D_MODEL], F32R, kind="ExternalInput")
    wk = nc.dram_tensor("wk", [P, D_MODEL], F32R, kind="ExternalInput")
    wv = nc.dram_tensor("wv", [P, D_MODEL], F32R, kind="ExternalInput")
    wo = nc.dram_tensor("wo", [P, KD * D_MODEL], F32R, kind="ExternalInput")
    ctab = nc.dram_tensor("ctab", [P, seq], F32, kind="ExternalInput")
    stab = nc.dram_tensor("stab", [P, seq], F32, kind="ExternalInput")
    dmaskd = nc.dram_tensor("dmask", [P, (CH // 128) * CH], F32,
                            kind="ExternalInput")
    ident = nc.dram_tensor("ident", [P, 128], F32, kind="ExternalInput")
    onesd = nc.dram_tensor("ones", [P, KB], F32, kind="ExternalInput")
    out_d = nc.dram_tensor("out", [D_MODEL, SW], F32, kind="ExternalOutput")

    with tile.TileContext(nc) as tc:
        with (
            tc.tile_pool(name="const", bufs=1) as cpool,
            tc.tile_pool(name="mats", bufs=1) as mpool,
            tc.tile_pool(name="xt", bufs=2) as xpool,
            tc.tile_pool(name="sc", bufs=2) as spool,
            tc.tile_pool(name="pt", bufs=4) as ptpool,
            tc.tile_pool(name="wo", bufs=3) as wopool,
            tc.tile_pool(name="ps", bufs=2, space="PSUM") as pspool,
            tc.tile_pool(name="pss", bufs=2, space="PSUM") as psspool,
            tc.tile_pool(name="dram", bufs=1, space="DRAM") as dpool,
        ):
            # ---- constants ----
            w_sb = {}
            for name, src in (("q", wq), ("k", wk), ("v", wv)):
                t = cpool.tile([P, D_MODEL], F32R, tag=f"w{name}")
                nc.sync.dma_start(out=t[:], in_=src[:])
                w_sb[name] = t
            ct = cpool.tile([P, seq], F32, tag="ct")
            nc.sync.dma_start(out=ct[:], in_=ctab[:])
            st = cpool.tile([P, seq], F32, tag="st")
            nc.sync.dma_start(out=st[:], in_=stab[:])
            dmask = cpool.tile([P, KBC * CH], F32, tag="dmask")
            nc.sync.dma_start(out=dmask[:], in_=dmaskd[:])
            idn = cpool.tile([P, 128], F32, tag="idn")
            nc.sync.dma_start(out=idn[:], in_=ident[:])
            ones = cpool.tile([P, KB], F32, tag="ones")
            nc.sync.dma_start(out=ones[:], in_=onesd[:])

            # ---- persistent matrices ----
            qT = mpool.tile([P, seq], F32R, tag="qT")  # rows: 2 heads x 64
            kT = mpool.tile([P, seq], F32R, tag="kT")
            vnat = mpool.tile([P, KB * 130], F32R, tag="vnat")

            a2a_in = dpool.tile([N_CORES, P, SW], F32R, tag="a2a_in")
            a2a_out = dpool.tile([N_CORES, P, SW], F32R, tag="a2a_out")

            def emit_proj_chunk(sc):
                """Projections + rope + V-transpose for seq chunk sc."""
                sl = bass.ts(sc, CH)
                xts = []
                for k in range(KD):
                    t = xpool.tile([P, CH], F32R, tag=f"xt{k}",
                                   name=f"xt_{sc}_{k}")
                    nc.sync.dma_start(
                        out=t[:], in_=xt[128 * k:128 * (k + 1), sl]
                    )
                    xts.append(t)
                vt_c = spool.tile([P, CH], F32, tag="vt")
                for name, dst in (("q", qT[:, sl]), ("k", kT[:, sl]),
                                  ("v", vt_c[:])):
                    ps = pspool.tile([P, CH], F32, tag="mm",
                                     name=f"proj_{sc}_{name}")
                    for k in range(KD):
                        nc.tensor.matmul(
                            ps[:],
                            w_sb[name][:, bass.ts(k, 128)],
                            xts[k][:],
                            start=(k == 0),
                            stop=(k == KD - 1),
                        )
                    nc.vector.tensor_copy(dst, ps[:])

                # rope on this chunk of qT and kT (in place)
                for mi, mat in ((0, qT), (1, kT)):
                    sw = spool.tile([P, CH], F32, tag="swap",
                                    name=f"swap_{sc}_{mi}")
                    for h in (0, 1):
                        for half in (0, 1):
                            d0 = 64 * h + 32 * half
                            s0 = 64 * h + 32 * (1 - half)
                            nc.vector.tensor_copy(
                                sw[d0:d0 + 32, :], mat[s0:s0 + 32, sl]
                            )
                    tm = spool.tile([P, CH], F32, tag="tmp",
                                    name=f"tmp_{sc}_{mi}")
                    nc.vector.tensor_mul(tm[:], sw[:], st[:, sl])
                    nc.vector.tensor_mul(mat[:, sl], mat[:, sl], ct[:, sl])
                    nc.vector.tensor_add(mat[:, sl], mat[:, sl], tm[:])

                # V transpose for this chunk's key blocks
                for j in range(KBC):
                    kb = sc * KBC + j
                    pst = psspool.tile([P, 2 * CH], F32, tag="s",
                                       name=f"vtr_{kb}")
                    nc.tensor.transpose(
                        pst[:, 0:128], vt_c[:, bass.ts(j, 128)], idn[:]
                    )
                    nc.vector.tensor_copy(
                        vnat[:, 130 * kb:130 * kb + 64], pst[:, 0:64]
                    )
                    nc.vector.tensor_copy(
                        vnat[:, 130 * kb + 65:130 * kb + 129],
                        pst[:, 64:128]
                    )

            def emit_attn_chunk(qc):
                """Attention for query chunk qc (needs proj chunks 0..qc).

                Per key block: S^T for both heads lands in one [128, 2*CH]
                PSUM tile ([0:CH]=h0, [CH:2CH]=h1) so a single wide exp
                covers both heads. Diagonal-band blocks are processed FIRST
                (their masking runs on GPSIMD and gets latency-hidden behind
                the non-diagonal tail of the PV accumulation).
                """
                kbmax = (qc + 1) * KBC
                psu = {}
                for h in (0, 1):
                    psu[h] = pspool.tile([65, CH], F32, tag="u",
                                         name=f"psu_{qc}_{h}")
                kb_order = (list(range(kbmax - KBC, kbmax))
                            + list(range(0, kbmax - KBC)))
                for ki, kb in enumerate(kb_order):
                    j = kb - (kbmax - KBC)  # diag index if >= 0
                    pss = psspool.tile([P, 2 * CH], F32, tag="s",
                                       name=f"sc_{qc}_{kb}")
                    for h in (0, 1):
                        nc.tensor.matmul(
                            pss[:, CH * h:CH * (h + 1)],
                            kT[64 * h:64 * (h + 1), bass.ts(kb, 128)],
                            qT[64 * h:64 * (h + 1), bass.ts(qc, CH)],
                            start=True,
                            stop=True,
                        )
                    pt = ptpool.tile([P, 2 * CH], F32R, tag="pt",
                                     name=f"pt_{qc}_{kb}")
                    nc.scalar.activation(pt[:], pss[:], EXP)
                    if j >= 0:
                        # zero q < key region: cols [0, 128j) fully +
                        # triangle at [128j, 128j+128), per head half
                        w = 128 * (j + 1)
                        for h in (0, 1):
                            nc.gpsimd.tensor_mul(
                                pt[:, CH * h:CH * h + w],
                                pt[:, CH * h:CH * h + w],
                                dmask[:, CH * j:CH * j + w],
                            )
                    for h in (0, 1):
                        nc.tensor.matmul(
                            psu[h][:],
                            vnat[:, 130 * kb + 65 * h:
                                 130 * kb + 65 * (h + 1)],
                            pt[:, CH * h:CH * (h + 1)],
                            start=(ki == 0),
                            stop=(ki == kbmax - 1),
                        )
                # epilogue: extract U and sums, normalize, stage a2a input
                for h in (0, 1):
                    ut = spool.tile([64, CH], F32R, tag=f"ut{h}",
                                    name=f"ut_{qc}_{h}")
                    nc.vector.tensor_copy(ut[:], psu[h][0:64, :])
                    ss = spool.tile([1, CH], F32, tag=f"ss{h}",
                                    name=f"ss_{qc}_{h}")
                    nc.vector.tensor_copy(ss[:], psu[h][64:65, :])
                    rs = spool.tile([1, CH], F32, tag=f"rs{h}",
                                    name=f"rs_{qc}_{h}")
                    nc.vector.reciprocal(rs[:], ss[:])
                    rb = spool.tile([64, CH], F32, tag=f"rb{h}",
                                    name=f"rb_{qc}_{h}")
                    nc.gpsimd.partition_broadcast(rb[:], rs[:])
                    nc.vector.tensor_mul(ut[:], ut[:], rb[:])
                    for jj in range(SPC):
                        shard = qc * SPC + jj
                        nc.sync.dma_start(
                            out=a2a_in[shard, 64 * h:64 * (h + 1), :],
                            in_=ut[:, SW * jj:SW * (jj + 1)],
                        )

            def emit_p12():
                # ones columns (cols 64 and 129 of each 130-wide block) via
                # strided DVE copies so the producer output dtype is f32r
                vv = vnat[:].rearrange("p (k c) -> p k c", c=130)
                oo = ones[:].rearrange("p (k c) -> p k c", c=1)
                nc.vector.tensor_copy(vv[:, :, 64:65], oo)
                nc.vector.tensor_copy(vv[:, :, 129:130], oo)
                for sc in range(NCH):
                    emit_proj_chunk(sc)
                    emit_attn_chunk(sc)

            def emit_p3():
                nc.gpsimd.collective_compute(
                    "AllToAll",
                    mybir.AluOpType.bypass,
                    replica_groups=[list(range(N_CORES))],
                    ins=[a2a_in.opt()],
                    outs=[a2a_out.opt()],
                )
                ats = []
                for i in range(N_CORES):
                    at = xpool.tile([P, SW], F32R, tag=f"xt{i}",
                                    name=f"at_{i}")
                    nc.sync.dma_start(out=at[:], in_=a2a_out[i])
                    ats.append(at)
                for e in range(KD):
                    wot = wopool.tile([P, D_MODEL], F32R, tag="wo",
                                      name=f"wot_{e}")
                    nc.sync.dma_start(
                        out=wot[:], in_=wo[:, bass.ts(e, D_MODEL)]
                    )
                    pso = pspool.tile([P, SW], F32, tag="mm",
                                      name=f"pso_{e}")
                    for i in range(N_CORES):
                        nc.tensor.matmul(
                            pso[:],
                            wot[:, bass.ts(i, 128)],
                            ats[i][:],
                            start=(i == 0),
                            stop=(i == N_CORES - 1),
                        )
                    ot = ptpool.tile([P, SW], F32, tag="pt",
                                     name=f"ot_{e}")
                    nc.vector.tensor_copy(ot[:], pso[:])
                    nc.sync.dma_start(out=out_d[bass.ts(e, 128)], in_=ot[:])

            if p12_reps == 1:
                emit_p12()
            else:
                with tc.For_i(0, p12_reps, 1):
                    emit_p12()
            for _ in range(p3_reps):
                emit_p3()

    nc.finalize()
    return nc


def prepare_in_maps(in_features, token_positions, Wq, Wk, Wv, Wo, seq):
    """Host-side staging: shard/transform full inputs into per-core maps."""
    x = np.ascontiguousarray(np.asarray(in_features, dtype=np.float32)[0])
    pos = np.asarray(token_positions).reshape(-1)[:seq].astype(np.float64)

    xt = np.ascontiguousarray(x.T)  # [D, S]

    # RoPE tables in rotate-half form after pair permutation.
    inv_freq = THETA ** (-np.arange(0, HEAD_DIM, 2, dtype=np.float64)
                         / HEAD_DIM)
    ang = pos[:, None] * inv_freq[None, :]  # [S, 32]
    cos = np.cos(ang).T.astype(np.float32)  # [32, S]
    sin = np.sin(ang).T.astype(np.float32)
    ctab = np.ascontiguousarray(np.tile(cos, (4, 1)))  # [128, S]
    stab = np.ascontiguousarray(
        np.concatenate([-sin, sin, -sin, sin], axis=0)
    ).astype(np.float32)

    perm = np.concatenate(
        [np.arange(0, HEAD_DIM, 2), np.arange(1, HEAD_DIM, 2)]
    )  # within-head: evens then odds

    CH = min(512, seq)
    KBC = CH // 128
    tri = np.triu(np.ones((128, 128), dtype=np.float32))
    dmask = np.ones((128, KBC * CH), dtype=np.float32)
    for j in range(KBC):
        dmask[:, CH * j:CH * j + 128 * j] = 0.0
        dmask[:, CH * j + 128 * j:CH * j + 128 * (j + 1)] = tri
    ident = np.eye(128, dtype=np.float32)
    ones = np.ones((128, seq // 128), dtype=np.float32)

    WoT = np.ascontiguousarray(np.asarray(Wo, dtype=np.float32).T)  # [d, e]
    wo_packed = np.empty((128, KD * D_MODEL), dtype=np.float32)
    for e in range(KD):
        for i in range(KD):
            wo_packed[:, D_MODEL * e + 128 * i: D_MODEL * e + 128 * (i + 1)] \
                = WoT[128 * i:128 * (i + 1), 128 * e:128 * (e + 1)]

    def pack_w(Wc):
        # Wc: [128 out, 1024 in] -> WT [1024, 128] -> [128, 8*128] k-tiled
        WT = np.ascontiguousarray(Wc.T)
        return np.ascontiguousarray(
            WT.reshape(KD, 128, 128).transpose(1, 0, 2).reshape(128, KD * 128)
        ).astype(np.float32)

    in_maps = []
    for c in range(N_CORES):
        rows = slice(128 * c, 128 * (c + 1))
        Wq_c = np.asarray(Wq, dtype=np.float32)[rows].reshape(2, 64, D_MODEL)
        Wq_c = (Wq_c[:, perm, :] / math.sqrt(HEAD_DIM)).reshape(128, D_MODEL)
        Wk_c = np.asarray(Wk, dtype=np.float32)[rows].reshape(2, 64, D_MODEL)
        Wk_c = Wk_c[:, perm, :].reshape(128, D_MODEL)
        Wv_c = np.asarray(Wv, dtype=np.float32)[rows]
        in_maps.append({
            "xt": xt,
            "wq": pack_w(Wq_c),
            "wk": pack_w(Wk_c),
            "wv": pack_w(Wv_c),
            "wo": wo_packed,
            "ctab": ctab,
            "stab": stab,
            "dmask": dmask,
            "ident": ident,
            "ones": ones,
        })
    return in_maps


_BUILD_CACHE = {}


def _get_nc(seq, p12_reps=1, p3_reps=1):
    key = (seq, p12_reps, p3_reps)
    if key not in _BUILD_CACHE:
        _BUILD_CACHE[key] = build(seq, p12_reps, p3_reps)
    return _BUILD_CACHE[key]


def postprocess(results, seq, in_dtype):
    SW = seq // N_CORES
    out = np.empty((seq, D_MODEL), dtype=np.float32)
    for c in range(N_CORES):
        out[SW * c:SW * (c + 1), :] = results[c]["out"].T
    return out.reshape(1, seq, D_MODEL).astype(in_dtype)


def kernel(in_features, token_positions, Wq, Wk, Wv, Wo):
    in_dtype = np.asarray(in_features).dtype
    B, S, D = np.asarray(in_features).shape
    assert B == 1 and D == D_MODEL

    nc = _get_nc(S)
    in_maps = prepare_in_maps(in_features, token_positions, Wq, Wk, Wv, Wo, S)
    res = run_bass_kernel_spmd(nc, in_maps, list(range(N_CORES)), trace=False)
    return postprocess(res.results, S, in_dtype)
